# revision 44
# baseline (speedup 1.0000x reference)
"""Trainium2 Bass kernel for nn_DenseBayesian (dense + hard LWTA grouped argmax mask).

Computes out = x @ W.T + b, then per group of U=4 output units keeps only the
argmax unit (others zeroed). Data-parallel over 8 NeuronCores along the row
axis; each core processes 32768 rows as 64 macro-tiles of 512 rows.

Matmul numerics (default scheme "c"): fp16 main product xh@wh plus e5m2 fp8
DoubleRow corrections pre-scaled so the products emerge at natural scale and
accumulate directly into the same PSUM group:

    u = xh@wh + (xl*2^8)@e5m2(wT/2^8) + e5m2(xh/2^8)@(wl*2^8)

This carries ~2^-15 effective logit precision (measured rel err 2.9e-3) at
16 matmul instructions per 512-row macro - the PE cycle floor, since fp8
DoubleRow on TRN2 doubles k per instruction but not MACs per cycle.

LWTA masking (default "mix"): W's columns are permuted host-side
(c = b0*256 + b1*128 + g for logical unit 2*b1+b0) so every compare partner
is a contiguous half-block - no strided SBUF access, and compare ties resolve
toward the lower index exactly like argmax. The scalar engine stages PSUM to
SBUF, then the DVE computes a 2-level max/is_ge tree. Alternating macros ship
either the full tree (winner f32 + 2 bit-planes, 1792B/128 rows/partition) or
just the pair level (pair maxes f32 + 1 bit-plane, 2560B) with the host doing
the final comparison - balancing DVE cycles against output DMA so both stay
under the PE roofline. The host unpacks bits and scatters winners into the
dense f32 [N, 512] output.

HW exec time: ~254 us on 8 NeuronCores (baseline dense-mask kernel: 499 us).

Self-contained: hardcodes the problem shapes; only needs numpy + the concourse
runtime available on the host.
"""
import numpy as np

import concourse.bass as bass
import concourse.mybir as mybir
import concourse.tile as tile
from concourse import bacc
from concourse.bass_utils import run_bass_kernel_spmd

f32 = mybir.dt.float32
f16 = mybir.dt.float16
f8 = mybir.dt.float8e4
f8e5 = mybir.dt.float8e5
u32 = mybir.dt.uint32

N = 262144
DIN = 256
DOUT = 512
U = 4
NCORES = 8
ROWS = N // NCORES          # 32768 rows per core
MACRO = 256                 # rows per macro-tile (2 psum banks of 128 rows)
P = 128
KC = DIN // P               # k chunks
G = DOUT // U               # groups per row (128)

SXL = 2.0 ** 11             # xl e4m3 pre-scale (variant b)
SWH = 2.0 ** 6              # wh e4m3 pre-scale (variant b)
SWL = 2.0 ** 17             # wl e4m3 pre-scale (variant b)
SCORR = 2.0 ** -17          # correction PSUM scale (variant b)
SC = 2.0 ** 8               # variant c e5m2 split scale (products at scale 1)

DR = mybir.MatmulPerfMode.DoubleRow


def _mask_and_store(nc, vpool, mpool, it, msk, ps, pk_dst, mask_mode):
    """v = (bits(u) & ~3) | lane_id; m2 = grouped max; DMA out."""
    v = vpool.tile([P, 2 * DOUT], f32)
    nc.vector.scalar_tensor_tensor(
        v[:].bitcast(u32), ps[:].bitcast(u32), msk[:, 0:1], it[:],
        op0=mybir.AluOpType.bitwise_and,
        op1=mybir.AluOpType.bitwise_or)
    m2 = mpool.tile([P, 2 * G], f32)
    if mask_mode == "reduce":
        nc.vector.tensor_reduce(
            m2[:], v[:].rearrange("p (g s) -> p g s", s=U),
            axis=mybir.AxisListType.X, op=mybir.AluOpType.max)
    else:
        vp = v[:].rearrange("p (a b) -> p a b", b=2)
        t1 = vpool.tile([P, DOUT], f32, tag="t1")
        nc.vector.tensor_tensor(t1[:], vp[:, :, 0], vp[:, :, 1],
                                mybir.AluOpType.max)
        tp = t1[:].rearrange("p (a b) -> p a b", b=2)
        nc.vector.tensor_tensor(m2[:], tp[:, :, 0], tp[:, :, 1],
                                mybir.AluOpType.max)
    nc.sync.dma_start(pk_dst, m2[:].rearrange("p (s g) -> p s g", s=2))


OB = 2 * G * 4 + DOUT + 2 * G   # 1792 bytes/partition: m2 f32 | d1 u8 | d2 u8


def _mask_cmp_store(nc, vpool, mpool, ps, o_dst, nrg):
    """Winner via compare tree: values + pair bits, no index embedding.

    Processed in independent halves (own Act stage, DVE ops, and DMA) so the
    drain chain at the end of the pipeline is half as deep and output DMA
    overlaps the remaining compare work. Ties resolve toward the lower
    index, matching argmax semantics. Per-half layout: m2 f32 | d1 | d2.
    """
    nh = 2 if nrg == 4 else 1            # halves per macro
    rgh = nrg // nh                      # row groups per half
    eh = rgh * DOUT                      # logits per partition per half
    obh = eh + eh // 2 + eh // 4
    for hh in range(nh):
        u = vpool.tile([P, eh], f32, tag=f"ucp{hh}")
        nc.scalar.activation(u[:], ps[:, hh * eh:(hh + 1) * eh],
                             mybir.ActivationFunctionType.Copy)
        us = u[:].rearrange("p (s c) -> p s c", c=DOUT)
        t1 = vpool.tile([P, eh // 2], f32, tag=f"t1{hh}")
        o = mpool.tile([P, obh], mybir.dt.uint8, tag=f"o{hh}")
        nc.vector.tensor_tensor(
            t1[:].rearrange("p (s c) -> p s c", c=DOUT // 2),
            us[:, :, 0:256], us[:, :, 256:512], mybir.AluOpType.max)
        nc.vector.tensor_tensor(
            o[:, eh:eh + eh // 2].rearrange("p (s c) -> p s c", c=DOUT // 2),
            us[:, :, 0:256], us[:, :, 256:512], mybir.AluOpType.is_ge)
        ts = t1[:].rearrange("p (s c) -> p s c", c=DOUT // 2)
        nc.vector.tensor_tensor(
            o[:, 0:eh].bitcast(f32).rearrange("p (s c) -> p s c",
                                              c=DOUT // 4),
            ts[:, :, 0:128], ts[:, :, 128:256], mybir.AluOpType.max)
        nc.vector.tensor_tensor(
            o[:, eh + eh // 2:obh].rearrange("p (s c) -> p s c",
                                             c=DOUT // 4),
            ts[:, :, 0:128], ts[:, :, 128:256], mybir.AluOpType.is_ge)
        nc.gpsimd.dma_start(o_dst[:, hh * obh:(hh + 1) * obh], o[:])


def _mask_t1_store(nc, vpool, mpool, ps, o_dst, nrg):
    """Minimal on-chip masking: pair maxes + pair bits; host does level 2.

    Same per-half staging as _mask_cmp_store. Per-half layout: t1 f32 | d1.
    """
    nh = 2 if nrg == 4 else 1
    rgh = nrg // nh
    eh = rgh * DOUT
    obh = eh * 2 + eh // 2
    for hh in range(nh):
        u = vpool.tile([P, eh], f32, tag=f"ucp{hh}")
        nc.scalar.activation(u[:], ps[:, hh * eh:(hh + 1) * eh],
                             mybir.ActivationFunctionType.Copy)
        us = u[:].rearrange("p (s c) -> p s c", c=DOUT)
        o = mpool.tile([P, obh], mybir.dt.uint8, tag=f"ot{hh}")
        nc.vector.tensor_tensor(
            o[:, 0:eh * 2].bitcast(f32).rearrange("p (s c) -> p s c",
                                                  c=DOUT // 2),
            us[:, :, 0:256], us[:, :, 256:512], mybir.AluOpType.max)
        nc.vector.tensor_tensor(
            o[:, eh * 2:obh].rearrange("p (s c) -> p s c", c=DOUT // 2),
            us[:, :, 0:256], us[:, :, 256:512], mybir.AluOpType.is_ge)
        nc.gpsimd.dma_start(o_dst[:, hh * obh:(hh + 1) * obh], o[:])


def build_program(n_macros: int, with_bias: bool, variant: str = "a"):
    """One NeuronCore program: n_macros macro-tiles of 256 rows each."""
    nc = bacc.Bacc("TRN2", target_bir_lowering=False)
    scheme = variant[0]                  # a | b | c
    macro = 512 if variant.endswith("5") else MACRO
    suffix = variant[:-1] if variant.endswith("5") else variant
    nrg = macro // P                     # 128-row groups per macro tile
    if suffix.endswith("_pt"):
        mask_mode = "pool_tree"
    elif suffix.endswith("_tree"):
        mask_mode = "dve_tree"
    elif suffix.endswith("_cmp"):
        mask_mode = "cmp"
    elif suffix.endswith("_t1"):
        mask_mode = "t1"
    elif suffix.endswith("_mix"):
        mask_mode = "mix"
    else:
        mask_mode = "reduce"
    x8dt = f8 if scheme == "b" else f8e5

    wh_d = nc.dram_tensor("wh", [P, KC, DOUT], f16, kind="ExternalInput")
    if scheme == "a":
        xh_d = nc.dram_tensor("xh", [n_macros, P, KC, MACRO], f16, kind="ExternalInput")
        xl_d = nc.dram_tensor("xl", [n_macros, P, KC, MACRO], f16, kind="ExternalInput")
        wl_d = nc.dram_tensor("wl", [P, KC, DOUT], f16, kind="ExternalInput")
    elif scheme == "c":
        xh_d = nc.dram_tensor("xh", [n_macros, P, KC, macro], f16, kind="ExternalInput")
        # piece 0 = xh8, piece 1 = xl8 (e5m2 bytes), one DMA per macro
        x8_d = nc.dram_tensor("x8", [n_macros, P, 2, KC, macro], mybir.dt.uint8,
                              kind="ExternalInput")
        wh8_d = nc.dram_tensor("wh8", [P, KC, DOUT], x8dt, kind="ExternalInput")
        wl8_d = nc.dram_tensor("wl8", [P, KC, DOUT], x8dt, kind="ExternalInput")
    else:
        xh_d = nc.dram_tensor("xh", [n_macros, P, KC, MACRO], f16, kind="ExternalInput")
        xh8_d = nc.dram_tensor("xh8", [n_macros, P, KC, MACRO], x8dt, kind="ExternalInput")
        xl8_d = nc.dram_tensor("xl8", [n_macros, P, KC, MACRO], x8dt, kind="ExternalInput")
        wh8_d = nc.dram_tensor("wh8", [P, KC, DOUT], x8dt, kind="ExternalInput")
        wl8_d = nc.dram_tensor("wl8", [P, KC, DOUT], x8dt, kind="ExternalInput")
    if with_bias:
        bh_d = nc.dram_tensor("bh", [1, DOUT], f16, kind="ExternalInput")
        bl_d = nc.dram_tensor("bl", [1, DOUT], f16, kind="ExternalInput")
    if mask_mode == "cmp":
        # m2 f32 | d1 u8 | d2 u8 packed per partition
        ob_d = nc.dram_tensor("pk", [n_macros, P, nrg * 896], mybir.dt.uint8,
                              kind="ExternalOutput")
    elif mask_mode == "t1":
        # t1 f32 | d1 u8 packed per partition
        ob_d = nc.dram_tensor("pk", [n_macros, P, nrg * 1280], mybir.dt.uint8,
                              kind="ExternalOutput")
    elif mask_mode == "mix":
        ob_d = nc.dram_tensor("pk", [(n_macros + 1) // 2, P, nrg * 1280],
                              mybir.dt.uint8, kind="ExternalOutput")
        ob2_d = nc.dram_tensor("pk2", [n_macros // 2, P, nrg * 896],
                               mybir.dt.uint8, kind="ExternalOutput")
    else:
        # packed winner (value with idx in low 2 bits), row = mt*256+s*128+p
        pk_d = nc.dram_tensor("pk", [n_macros, P, nrg, G], f32,
                              kind="ExternalOutput")

    psm_bufs = (8 // nrg) if scheme == "c" else 2
    with tile.TileContext(nc) as tc:
        with tc.tile_pool(name="wpool", bufs=1) as wpool, \
             tc.tile_pool(name="xpool", bufs=4) as xpool, \
             tc.tile_pool(name="cspool", bufs=3) as cspool, \
             tc.tile_pool(name="vpool", bufs=3) as vpool, \
             tc.tile_pool(name="mpool", bufs=3) as mpool, \
             tc.tile_pool(name="psm", bufs=psm_bufs, space="PSUM") as psm, \
             tc.tile_pool(name="psc", bufs=2, space="PSUM") as psc:

            wh = wpool.tile([P, KC, DOUT], f16)
            nc.sync.dma_start(wh[:], wh_d[:])
            if scheme == "a":
                wl = wpool.tile([P, KC, DOUT], f16)
                nc.sync.dma_start(wl[:], wl_d[:])
            else:
                wh8 = wpool.tile([P, KC, DOUT], x8dt)
                nc.scalar.dma_start(wh8[:], wh8_d[:])
                wl8 = wpool.tile([P, KC, DOUT], x8dt)
                nc.gpsimd.dma_start(wl8[:], wl8_d[:])
            if scheme == "b":
                # fp16 identity for folding the correction PSUM into main
                ident = wpool.tile([P, P], f16)
                nc.gpsimd.memset(ident[:], 1.0)
                nc.gpsimd.affine_select(
                    ident[:], ident[:], pattern=[[-1, P]],
                    compare_op=mybir.AluOpType.is_equal, fill=0.0,
                    base=0, channel_multiplier=1)
            if with_bias:
                bh = wpool.tile([1, DOUT], f16)
                nc.sync.dma_start(bh[:], bh_d[:])
                bl = wpool.tile([1, DOUT], f16)
                nc.sync.dma_start(bl[:], bl_d[:])
                ones = wpool.tile([1, P], f16)
                nc.gpsimd.memset(ones[:], 1.0)

            if mask_mode not in ("cmp", "t1", "mix"):
                # lane-id pattern 0,1,2,3 repeating + the ~3 AND-mask scalar
                it = wpool.tile([P, 2 * DOUT], u32)
                nc.gpsimd.iota(it[:], pattern=[[0, 2 * G], [1, U]], base=0,
                               channel_multiplier=0)
                msk = wpool.tile([P, 1], u32)
                nc.vector.memset(msk[:], 0xFFFFFFFC)
            else:
                it = msk = None

            for mt in range(n_macros):
                if scheme == "a":
                    xh_t = xpool.tile([P, KC, MACRO], f16, tag="xh")
                    nc.sync.dma_start(xh_t[:], xh_d[mt, :, :, :])
                    xl_t = xpool.tile([P, KC, MACRO], f16, tag="xl")
                    nc.sync.dma_start(xl_t[:], xl_d[mt, :, :, :])

                    ps = psm.tile([P, 2 * DOUT], f32)
                    for s in range(2):
                        acc = ps[:, s * DOUT:(s + 1) * DOUT]
                        mms = []
                        if with_bias:
                            mms.append((ones[:, :], bh[:, :]))
                            mms.append((ones[:, :], bl[:, :]))
                        rs = slice(s * P, (s + 1) * P)
                        for (xa, wb) in ((xl_t, wh), (xh_t, wl), (xh_t, wh)):
                            for c in range(KC):
                                mms.append((xa[:, c, rs], wb[:, c, :]))
                        last = len(mms) - 1
                        for i, (lhsT, rhs) in enumerate(mms):
                            nc.tensor.matmul(acc, lhsT, rhs,
                                             start=(i == 0), stop=(i == last))
                elif scheme == "c":
                    xh_t = xpool.tile([P, KC, macro], f16, tag="xh")
                    nc.sync.dma_start(xh_t[:], xh_d[mt, :, :, :])
                    x8_t = xpool.tile([P, 2, KC, macro], mybir.dt.uint8,
                                      tag="x8")
                    nc.gpsimd.dma_start(x8_t[:], x8_d[mt])
                    xh8_t = x8_t[:, 0].bitcast(x8dt)
                    xl8_t = x8_t[:, 1].bitcast(x8dt)

                    ps = psm.tile([P, nrg * DOUT], f32, tag="ps")
                    # all fp16 mains first, then all fp8 DR corrections:
                    # dtype switches flush the PE pipeline, so keep runs long
                    for s in range(nrg):
                        acc = ps[:, s * DOUT:(s + 1) * DOUT]
                        rs = slice(s * P, (s + 1) * P)
                        first = True
                        if with_bias:
                            nc.tensor.matmul(acc, ones[:, :], bh[:, :],
                                             start=True, stop=False)
                            nc.tensor.matmul(acc, ones[:, :], bl[:, :],
                                             start=False, stop=False)
                            first = False
                        for c in range(KC):
                            nc.tensor.matmul(acc, xh_t[:, c, rs], wh[:, c, :],
                                             start=first, stop=False)
                            first = False
                    for s in range(nrg):
                        acc = ps[:, s * DOUT:(s + 1) * DOUT]
                        rs = slice(s * P, (s + 1) * P)
                        # e5m2 corrections at natural scale, same accumulator
                        nc.tensor.matmul(acc, xl8_t[:, :, rs], wh8[:, :, :],
                                         start=False, stop=False, perf_mode=DR)
                        nc.tensor.matmul(acc, xh8_t[:, :, rs], wl8[:, :, :],
                                         start=False, stop=True, perf_mode=DR)
                else:
                    xh_t = xpool.tile([P, KC, MACRO], f16, tag="xh")
                    nc.sync.dma_start(xh_t[:], xh_d[mt, :, :, :])
                    xh8_t = xpool.tile([P, KC, MACRO], f8, tag="xh8")
                    nc.sync.dma_start(xh8_t[:], xh8_d[mt, :, :, :])
                    xl8_t = xpool.tile([P, KC, MACRO], f8, tag="xl8")
                    nc.sync.dma_start(xl8_t[:], xl8_d[mt, :, :, :])

                    cps = psc.tile([P, 2 * DOUT], f32)
                    ps = psm.tile([P, 2 * DOUT], f32)
                    # fp8 DoubleRow corrections first so the scalar engine can
                    # rescale them while the PE runs the fp16 main products
                    for s in range(2):
                        rs = slice(s * P, (s + 1) * P)
                        cacc = cps[:, s * DOUT:(s + 1) * DOUT]
                        nc.tensor.matmul(cacc, xl8_t[:, :, rs], wh8[:, :, :],
                                         start=True, stop=False, perf_mode=DR)
                        nc.tensor.matmul(cacc, xh8_t[:, :, rs], wl8[:, :, :],
                                         start=False, stop=True, perf_mode=DR)
                    csb = cspool.tile([P, 2, DOUT], f16)
                    for s in range(2):
                        nc.scalar.activation(
                            csb[:, s, :], cps[:, s * DOUT:(s + 1) * DOUT],
                            mybir.ActivationFunctionType.Copy, scale=SCORR)
                    for s in range(2):
                        rs = slice(s * P, (s + 1) * P)
                        acc = ps[:, s * DOUT:(s + 1) * DOUT]
                        first = True
                        if with_bias:
                            nc.tensor.matmul(acc, ones[:, :], bh[:, :],
                                             start=True, stop=False)
                            nc.tensor.matmul(acc, ones[:, :], bl[:, :],
                                             start=False, stop=False)
                            first = False
                        for c in range(KC):
                            nc.tensor.matmul(acc, xh_t[:, c, rs], wh[:, c, :],
                                             start=first, stop=False)
                            first = False
                    for s in range(2):
                        acc = ps[:, s * DOUT:(s + 1) * DOUT]
                        nc.tensor.matmul(acc, ident[:], csb[:, s, :],
                                         start=False, stop=True)

                if mask_mode == "cmp":
                    _mask_cmp_store(nc, vpool, mpool, ps, ob_d[mt], nrg)
                elif mask_mode == "t1":
                    _mask_t1_store(nc, vpool, mpool, ps, ob_d[mt], nrg)
                elif mask_mode == "mix":
                    if mt % 2 == 0:
                        _mask_t1_store(nc, vpool, mpool, ps, ob_d[mt // 2], nrg)
                    else:
                        _mask_cmp_store(nc, vpool, mpool, ps, ob2_d[mt // 2], nrg)
                else:
                    _mask_and_store(nc, vpool, mpool, it, msk, ps, pk_d[mt],
                                    mask_mode)

    nc.compile()
    return nc


_programs: dict = {}


def _get_program(n_macros: int, with_bias: bool, variant: str = "a"):
    key = (n_macros, with_bias, variant)
    if key not in _programs:
        _programs[key] = build_program(n_macros, with_bias, variant)
    return _programs[key]


def _split_fp16(a: np.ndarray):
    hi = a.astype(np.float16)
    lo = (a - hi.astype(np.float32)).astype(np.float16)
    return hi, lo


def _pack_b(b: np.ndarray):
    """[DOUT] fp32 -> (hi, lo) [1, DOUT] fp16."""
    return _split_fp16(np.ascontiguousarray(b.astype(np.float32).reshape(1, DOUT)))


def _tile_x(a: np.ndarray, n_macros: int, macro: int = MACRO) -> np.ndarray:
    """[rows, DIN] -> [n_macros, P, KC, macro] keeping dtype."""
    at = np.ascontiguousarray(a.T)                      # [DIN, rows]
    at = at.reshape(KC, P, n_macros, macro)             # [c, p, mt, r]
    return np.ascontiguousarray(at.transpose(2, 1, 0, 3))


def _pack_x(xs: np.ndarray, n_macros: int):
    """[rows, DIN] fp32 -> (hi, lo) tiled [n_macros, P, KC, MACRO] fp16."""
    hi, lo = _split_fp16(xs)
    return [_tile_x(a, n_macros) for a in (hi, lo)]


def _pack_x8(xs: np.ndarray, n_macros: int):
    """fp32 rows -> (xh fp16, xh8 fp8, xl8 fp8 scaled) tiles for variant b."""
    f8np = mybir.dt.np(f8)
    hi = xs.astype(np.float16)
    lo32 = xs - hi.astype(np.float32)
    xh = _tile_x(hi, n_macros)
    xh8 = _tile_x(hi.astype(f8np), n_macros)
    xl8 = _tile_x((lo32 * SXL).astype(f8np), n_macros)
    return xh, xh8, xl8


def _pack_x8e5(xs: np.ndarray, n_macros: int, macro: int = MACRO):
    """fp32 rows -> (xh fp16, x8 packed u8 [nm, P, 2, KC, MACRO]), variant c.

    x8 piece 0 = e5m2(xh / 2^8), piece 1 = e5m2(xl * 2^8).
    """
    e5 = mybir.dt.np(f8e5)
    hi = xs.astype(np.float16)
    lo32 = xs - hi.astype(np.float32)
    xh = _tile_x(hi, n_macros, macro)
    xh8 = _tile_x((hi.astype(np.float32) / SC).astype(e5), n_macros, macro)
    xl8 = _tile_x((lo32 * SC).astype(e5), n_macros, macro)
    x8 = np.stack([xh8.view(np.uint8), xl8.view(np.uint8)], axis=2)
    return xh, np.ascontiguousarray(x8)


# physical output column c -> logical unit index (4*g + u), u = 2*b1 + b0,
# c = b0*256 + b1*128 + g: makes all LWTA compare partners contiguous blocks
LPERM = np.array([4 * (c & 127) + 2 * ((c >> 7) & 1) + (c >> 8)
                  for c in range(DOUT)])


def _pack_w8e5(W: np.ndarray, perm: bool = False):
    """-> (wh fp16, wh8 e5m2/2^8, wl8 e5m2*2^8) tiles, variant c."""
    e5 = mybir.dt.np(f8e5)
    wT = np.ascontiguousarray(W.astype(np.float32).T)   # [DIN, DOUT]
    if perm:
        wT = np.ascontiguousarray(wT[:, LPERM])
    hi = wT.astype(np.float16)
    lo32 = wT - hi.astype(np.float32)
    wh = _tile_w(hi)
    wh8 = _tile_w((wT / SC).astype(e5))
    wl8 = _tile_w((lo32 * SC).astype(e5))
    return wh, wh8, wl8


def _tile_w(a: np.ndarray) -> np.ndarray:
    """[DIN, DOUT] -> [P, KC, DOUT] keeping dtype."""
    return np.ascontiguousarray(a.reshape(KC, P, DOUT).transpose(1, 0, 2))


def _pack_w(W: np.ndarray):
    """[DOUT, DIN] fp32 -> (hi, lo) tiled [P, KC, DOUT] fp16 of W.T."""
    wT = W.astype(np.float32).T                         # [DIN, DOUT]
    hi, lo = _split_fp16(np.ascontiguousarray(wT))
    return [_tile_w(a) for a in (hi, lo)]


def _pack_w8(W: np.ndarray):
    """-> (wh fp16, wh8 fp8 * 2^6, wl8 fp8 * 2^17) tiles for variant b."""
    f8np = mybir.dt.np(f8)
    wT = np.ascontiguousarray(W.astype(np.float32).T)   # [DIN, DOUT]
    hi = wT.astype(np.float16)
    lo32 = wT - hi.astype(np.float32)
    wh = _tile_w(hi)
    wh8 = _tile_w((wT * SWH).astype(f8np))
    wl8 = _tile_w((lo32 * SWL).astype(f8np))
    return wh, wh8, wl8


def _unpack_result_map(rm: dict, rows: int) -> np.ndarray:
    """Unpack one core's result dict (handles the mix-mode tensor pair)."""
    if "pk2" in rm:
        t1, cm = rm["pk"], rm["pk2"]
        nm = t1.shape[0] + cm.shape[0]
        macro = rows // nm
        a = _unpack_out_t1(t1, t1.shape[0] * macro)
        b = _unpack_out_cmp(cm, cm.shape[0] * macro)
        out = np.empty((nm, macro, DOUT), np.float32)
        out[0::2] = a.reshape(t1.shape[0], macro, DOUT)
        out[1::2] = b.reshape(cm.shape[0], macro, DOUT)
        return out.reshape(rows, DOUT)
    return _unpack_out(rm["pk"], rows)


def _unpack_out(pk: np.ndarray, rows: int) -> np.ndarray:
    """Device output -> dense [rows, DOUT] f32 (bit-packed, cmp, t1 format)."""
    if pk.dtype == np.uint8:
        if pk.shape[2] % 1280 == 0:
            return _unpack_out_t1(pk, rows)
        return _unpack_out_cmp(pk, rows)
    # row = mt*256 + s*128 + p  ->  [mt, s, p, g]
    m2 = np.ascontiguousarray(pk.transpose(0, 2, 1, 3)).reshape(rows, G)
    iv = m2.view(np.uint32)
    idx = (iv & np.uint32(3)).astype(np.int64)
    val = (iv & np.uint32(0xFFFFFFFC)).view(np.float32)
    out = np.zeros((rows, G, U), dtype=np.float32)
    np.put_along_axis(out, idx[:, :, None], val[:, :, None], axis=2)
    return out.reshape(rows, DOUT)


def _unpack_out_t1(ob: np.ndarray, rows: int) -> np.ndarray:
    """[n_macros, P, nh*(t1 f32 | d1 u8)] -> [rows, DOUT].

    Per half-block order (s_local, b1, g); host computes the level-2 max.
    """
    nm = ob.shape[0]
    nrg = ob.shape[2] // 1280
    nh = 2 if nrg == 4 else 1
    rgh = nrg // nh
    eh = rgh * DOUT
    blk = ob.reshape(nm, P, nh, eh * 2 + eh // 2)
    t1 = blk[:, :, :, 0:eh * 2].copy().view(np.float32)
    t1 = t1.reshape(nm, P, nrg, 2, G)
    d1 = (blk[:, :, :, eh * 2:] != 0).reshape(nm, P, nrg, 2, G)
    b1w = np.where(t1[:, :, :, 0, :] >= t1[:, :, :, 1, :], 0, 1)
    val = np.maximum(t1[:, :, :, 0, :], t1[:, :, :, 1, :])
    b0w = np.where(np.take_along_axis(d1, b1w[:, :, :, None, :],
                                      axis=3)[:, :, :, 0, :], 0, 1)
    idx = (2 * b1w + b0w).astype(np.int64)
    val = val.transpose(0, 2, 1, 3).reshape(rows, G)
    idx = idx.transpose(0, 2, 1, 3).reshape(rows, G)
    out = np.zeros((rows, G, U), dtype=np.float32)
    np.put_along_axis(out, idx[:, :, None], val[:, :, None], axis=2)
    return out.reshape(rows, DOUT)


def _unpack_out_cmp(ob: np.ndarray, rows: int) -> np.ndarray:
    """[n_macros, P, nh*(m2 f32 | d1 u8 | d2 u8)] -> [rows, DOUT].

    Per half-block: d1 order (s_local, b1, g), m2/d2 order (s_local, g); a
    set compare bit means the first (lower-index) operand won.
    """
    nm = ob.shape[0]
    nrg = ob.shape[2] // 896
    nh = 2 if nrg == 4 else 1
    rgh = nrg // nh
    eh = rgh * DOUT
    blk = ob.reshape(nm, P, nh, eh + eh // 2 + eh // 4)
    val = blk[:, :, :, 0:eh].copy().view(np.float32).reshape(nm, P, nrg, G)
    d1 = (blk[:, :, :, eh:eh + eh // 2] != 0).reshape(nm, P, nrg, 2, G)
    d2 = (blk[:, :, :, eh + eh // 2:] != 0).reshape(nm, P, nrg, G)
    b1w = np.where(d2, 0, 1)
    b0w = np.where(np.take_along_axis(d1, b1w[:, :, :, None, :],
                                      axis=3)[:, :, :, 0, :], 0, 1)
    idx = (2 * b1w + b0w).astype(np.int64)
    # row = mt*macro + s*128 + p
    val = val.transpose(0, 2, 1, 3).reshape(rows, G)
    idx = idx.transpose(0, 2, 1, 3).reshape(rows, G)
    out = np.zeros((rows, G, U), dtype=np.float32)
    np.put_along_axis(out, idx[:, :, None], val[:, :, None], axis=2)
    return out.reshape(rows, DOUT)


def _build_in_maps(x, W, b, with_bias, n_macros, variant):
    in_maps = []
    scheme = variant[0]
    if scheme == "a":
        wh, wl = _pack_w(W)
        for i in range(NCORES):
            xh, xl = _pack_x(x[i * ROWS:(i + 1) * ROWS], n_macros)
            im = {"xh": xh, "xl": xl, "wh": wh, "wl": wl}
            if with_bias:
                im["bh"], im["bl"] = _pack_b(b)
            in_maps.append(im)
    elif scheme == "b":
        wh, wh8, wl8 = _pack_w8(W)
        for i in range(NCORES):
            xh, xh8, xl8 = _pack_x8(x[i * ROWS:(i + 1) * ROWS], n_macros)
            im = {"xh": xh, "xh8": xh8, "xl8": xl8,
                  "wh": wh, "wh8": wh8, "wl8": wl8}
            if with_bias:
                im["bh"], im["bl"] = _pack_b(b)
            in_maps.append(im)
    else:
        macro = 512 if variant.endswith("5") else MACRO
        n_macros = ROWS // macro
        perm = ("_cmp" in variant or "_t1" in variant
                or "_mix" in variant)
        bp = b[LPERM] if perm else b
        wh, wh8, wl8 = _pack_w8e5(W, perm=perm)
        for i in range(NCORES):
            xh, x8 = _pack_x8e5(x[i * ROWS:(i + 1) * ROWS], n_macros, macro)
            im = {"xh": xh, "x8": x8, "wh": wh, "wh8": wh8, "wl8": wl8}
            if with_bias:
                im["bh"], im["bl"] = _pack_b(bp)
            in_maps.append(im)
    return in_maps


VARIANT = "c_mix5"


def n_macros_for(variant: str) -> int:
    return ROWS // (512 if variant.endswith("5") else MACRO)


def kernel(x: np.ndarray, W: np.ndarray, b: np.ndarray) -> np.ndarray:
    x = np.asarray(x, dtype=np.float32)
    W = np.asarray(W, dtype=np.float32)
    b = np.asarray(b, dtype=np.float32)
    assert x.shape == (N, DIN) and W.shape == (DOUT, DIN) and b.shape == (DOUT,)

    with_bias = bool(np.any(b))
    n_macros = n_macros_for(VARIANT)
    nc = _get_program(n_macros, with_bias, VARIANT)

    in_maps = _build_in_maps(x, W, b, with_bias, n_macros, VARIANT)
    res = run_bass_kernel_spmd(nc, in_maps, list(range(NCORES)))
    return np.concatenate(
        [_unpack_result_map(res.results[i], ROWS) for i in range(NCORES)], axis=0)


# revision 45
# speedup vs baseline: 1.0294x; 1.0294x over previous
"""Trainium2 Bass kernel for nn_DenseBayesian (dense + hard LWTA grouped argmax mask).

Computes out = x @ W.T + b, then per group of U=4 output units keeps only the
argmax unit (others zeroed). Data-parallel over 8 NeuronCores along the row
axis; each core processes 32768 rows as 64 macro-tiles of 512 rows.

Matmul numerics (default scheme "c"): fp16 main product xh@wh plus e5m2 fp8
DoubleRow corrections pre-scaled so the products emerge at natural scale and
accumulate directly into the same PSUM group:

    u = xh@wh + (xl*2^8)@e5m2(wT/2^8) + e5m2(xh/2^8)@(wl*2^8)

This carries ~2^-15 effective logit precision (measured rel err 2.9e-3) at
16 matmul instructions per 512-row macro - the PE cycle floor, since fp8
DoubleRow on TRN2 doubles k per instruction but not MACs per cycle.

LWTA masking (default "mix"): W's columns are permuted host-side
(c = b0*256 + b1*128 + g for logical unit 2*b1+b0) so every compare partner
is a contiguous half-block - no strided SBUF access, and compare ties resolve
toward the lower index exactly like argmax. The scalar engine stages PSUM to
SBUF, then the DVE computes a 2-level max/is_ge tree. Alternating macros ship
either the full tree (winner f32 + 2 bit-planes, 1792B/128 rows/partition) or
just the pair level (pair maxes f32 + 1 bit-plane, 2560B) with the host doing
the final comparison - balancing DVE cycles against output DMA so both stay
under the PE roofline. The host unpacks bits and scatters winners into the
dense f32 [N, 512] output.

HW exec time: ~254 us on 8 NeuronCores (baseline dense-mask kernel: 499 us).

Self-contained: hardcodes the problem shapes; only needs numpy + the concourse
runtime available on the host.
"""
import numpy as np

import concourse.bass as bass
import concourse.mybir as mybir
import concourse.tile as tile
from concourse import bacc
from concourse.bass_utils import run_bass_kernel_spmd

f32 = mybir.dt.float32
f16 = mybir.dt.float16
f8 = mybir.dt.float8e4
f8e5 = mybir.dt.float8e5
u32 = mybir.dt.uint32

N = 262144
DIN = 256
DOUT = 512
U = 4
NCORES = 8
ROWS = N // NCORES          # 32768 rows per core
MACRO = 256                 # rows per macro-tile (2 psum banks of 128 rows)
P = 128
KC = DIN // P               # k chunks
G = DOUT // U               # groups per row (128)

SXL = 2.0 ** 11             # xl e4m3 pre-scale (variant b)
SWH = 2.0 ** 6              # wh e4m3 pre-scale (variant b)
SWL = 2.0 ** 17             # wl e4m3 pre-scale (variant b)
SCORR = 2.0 ** -17          # correction PSUM scale (variant b)
SC = 2.0 ** 8               # variant c e5m2 split scale (products at scale 1)

DR = mybir.MatmulPerfMode.DoubleRow


def _mask_and_store(nc, vpool, mpool, it, msk, ps, pk_dst, mask_mode):
    """v = (bits(u) & ~3) | lane_id; m2 = grouped max; DMA out."""
    v = vpool.tile([P, 2 * DOUT], f32)
    nc.vector.scalar_tensor_tensor(
        v[:].bitcast(u32), ps[:].bitcast(u32), msk[:, 0:1], it[:],
        op0=mybir.AluOpType.bitwise_and,
        op1=mybir.AluOpType.bitwise_or)
    m2 = mpool.tile([P, 2 * G], f32)
    if mask_mode == "reduce":
        nc.vector.tensor_reduce(
            m2[:], v[:].rearrange("p (g s) -> p g s", s=U),
            axis=mybir.AxisListType.X, op=mybir.AluOpType.max)
    else:
        vp = v[:].rearrange("p (a b) -> p a b", b=2)
        t1 = vpool.tile([P, DOUT], f32, tag="t1")
        nc.vector.tensor_tensor(t1[:], vp[:, :, 0], vp[:, :, 1],
                                mybir.AluOpType.max)
        tp = t1[:].rearrange("p (a b) -> p a b", b=2)
        nc.vector.tensor_tensor(m2[:], tp[:, :, 0], tp[:, :, 1],
                                mybir.AluOpType.max)
    nc.sync.dma_start(pk_dst, m2[:].rearrange("p (s g) -> p s g", s=2))


OB = 2 * G * 4 + DOUT + 2 * G   # 1792 bytes/partition: m2 f32 | d1 u8 | d2 u8


def _mask_cmp_store(nc, vpool, mpool, ps, o_dst, nrg):
    """Winner via compare tree: values + pair bits, no index embedding.

    Processed in independent halves (own Act stage, DVE ops, and DMA) so the
    drain chain at the end of the pipeline is half as deep and output DMA
    overlaps the remaining compare work. Ties resolve toward the lower
    index, matching argmax semantics. Per-half layout: m2 f32 | d1 | d2.
    """
    nh = 2 if nrg == 4 else 1            # halves per macro
    rgh = nrg // nh                      # row groups per half
    eh = rgh * DOUT                      # logits per partition per half
    obh = eh + eh // 2 + eh // 4
    for hh in range(nh):
        u = vpool.tile([P, eh], f32, tag=f"ucp{hh}")
        nc.scalar.activation(u[:], ps[:, hh * eh:(hh + 1) * eh],
                             mybir.ActivationFunctionType.Copy)
        us = u[:].rearrange("p (s c) -> p s c", c=DOUT)
        t1 = vpool.tile([P, eh // 2], f32, tag=f"t1{hh}")
        o = mpool.tile([P, obh], mybir.dt.uint8, tag=f"o{hh}")
        nc.vector.tensor_tensor(
            t1[:].rearrange("p (s c) -> p s c", c=DOUT // 2),
            us[:, :, 0:256], us[:, :, 256:512], mybir.AluOpType.max)
        nc.vector.tensor_tensor(
            o[:, eh:eh + eh // 2].rearrange("p (s c) -> p s c", c=DOUT // 2),
            us[:, :, 0:256], us[:, :, 256:512], mybir.AluOpType.is_ge)
        ts = t1[:].rearrange("p (s c) -> p s c", c=DOUT // 2)
        nc.vector.tensor_tensor(
            o[:, 0:eh].bitcast(f32).rearrange("p (s c) -> p s c",
                                              c=DOUT // 4),
            ts[:, :, 0:128], ts[:, :, 128:256], mybir.AluOpType.max)
        nc.vector.tensor_tensor(
            o[:, eh + eh // 2:obh].rearrange("p (s c) -> p s c",
                                             c=DOUT // 4),
            ts[:, :, 0:128], ts[:, :, 128:256], mybir.AluOpType.is_ge)
        nc.gpsimd.dma_start(o_dst[:, hh * obh:(hh + 1) * obh], o[:])


def _mask_t1_store(nc, vpool, mpool, ps, o_dst, nrg):
    """Minimal on-chip masking: pair maxes + pair bits; host does level 2.

    Same per-half staging as _mask_cmp_store. Per-half layout: t1 f32 | d1.
    """
    nh = 2 if nrg == 4 else 1
    rgh = nrg // nh
    eh = rgh * DOUT
    obh = eh * 2 + eh // 2
    for hh in range(nh):
        u = vpool.tile([P, eh], f32, tag=f"ucp{hh}")
        nc.scalar.activation(u[:], ps[:, hh * eh:(hh + 1) * eh],
                             mybir.ActivationFunctionType.Copy)
        us = u[:].rearrange("p (s c) -> p s c", c=DOUT)
        o = mpool.tile([P, obh], mybir.dt.uint8, tag=f"ot{hh}")
        nc.vector.tensor_tensor(
            o[:, 0:eh * 2].bitcast(f32).rearrange("p (s c) -> p s c",
                                                  c=DOUT // 2),
            us[:, :, 0:256], us[:, :, 256:512], mybir.AluOpType.max)
        nc.vector.tensor_tensor(
            o[:, eh * 2:obh].rearrange("p (s c) -> p s c", c=DOUT // 2),
            us[:, :, 0:256], us[:, :, 256:512], mybir.AluOpType.is_ge)
        nc.gpsimd.dma_start(o_dst[:, hh * obh:(hh + 1) * obh], o[:])


def build_program(n_macros: int, with_bias: bool, variant: str = "a"):
    """One NeuronCore program: n_macros macro-tiles of 256 rows each."""
    nc = bacc.Bacc("TRN2", target_bir_lowering=False)
    scheme = variant[0]                  # a | b | c
    macro = 512 if variant.endswith("5") else MACRO
    suffix = variant[:-1] if variant.endswith("5") else variant
    nrg = macro // P                     # 128-row groups per macro tile
    if suffix.endswith("_pt"):
        mask_mode = "pool_tree"
    elif suffix.endswith("_tree"):
        mask_mode = "dve_tree"
    elif suffix.endswith("_cmp"):
        mask_mode = "cmp"
    elif suffix.endswith("_t1"):
        mask_mode = "t1"
    elif suffix.endswith("_mix"):
        mask_mode = "mix"
    else:
        mask_mode = "reduce"
    x8dt = f8 if scheme == "b" else f8e5

    wh_d = nc.dram_tensor("wh", [P, KC, DOUT], f16, kind="ExternalInput")
    if scheme == "a":
        xh_d = nc.dram_tensor("xh", [n_macros, P, KC, MACRO], f16, kind="ExternalInput")
        xl_d = nc.dram_tensor("xl", [n_macros, P, KC, MACRO], f16, kind="ExternalInput")
        wl_d = nc.dram_tensor("wl", [P, KC, DOUT], f16, kind="ExternalInput")
    elif scheme == "c":
        xh_d = nc.dram_tensor("xh", [n_macros, P, KC, macro], f16, kind="ExternalInput")
        # piece 0 = xh8, piece 1 = xl8 (e5m2 bytes), one DMA per macro
        x8_d = nc.dram_tensor("x8", [n_macros, P, 2, KC, macro], mybir.dt.uint8,
                              kind="ExternalInput")
        wh8_d = nc.dram_tensor("wh8", [P, KC, DOUT], x8dt, kind="ExternalInput")
        wl8_d = nc.dram_tensor("wl8", [P, KC, DOUT], x8dt, kind="ExternalInput")
    else:
        xh_d = nc.dram_tensor("xh", [n_macros, P, KC, MACRO], f16, kind="ExternalInput")
        xh8_d = nc.dram_tensor("xh8", [n_macros, P, KC, MACRO], x8dt, kind="ExternalInput")
        xl8_d = nc.dram_tensor("xl8", [n_macros, P, KC, MACRO], x8dt, kind="ExternalInput")
        wh8_d = nc.dram_tensor("wh8", [P, KC, DOUT], x8dt, kind="ExternalInput")
        wl8_d = nc.dram_tensor("wl8", [P, KC, DOUT], x8dt, kind="ExternalInput")
    if with_bias:
        bh_d = nc.dram_tensor("bh", [1, DOUT], f16, kind="ExternalInput")
        bl_d = nc.dram_tensor("bl", [1, DOUT], f16, kind="ExternalInput")
    if mask_mode == "cmp":
        # m2 f32 | d1 u8 | d2 u8 packed per partition
        ob_d = nc.dram_tensor("pk", [n_macros, P, nrg * 896], mybir.dt.uint8,
                              kind="ExternalOutput")
    elif mask_mode == "t1":
        # t1 f32 | d1 u8 packed per partition
        ob_d = nc.dram_tensor("pk", [n_macros, P, nrg * 1280], mybir.dt.uint8,
                              kind="ExternalOutput")
    elif mask_mode == "mix":
        ob_d = nc.dram_tensor("pk", [(n_macros + 1) // 2, P, nrg * 1280],
                              mybir.dt.uint8, kind="ExternalOutput")
        ob2_d = nc.dram_tensor("pk2", [n_macros // 2, P, nrg * 896],
                               mybir.dt.uint8, kind="ExternalOutput")
    else:
        # packed winner (value with idx in low 2 bits), row = mt*256+s*128+p
        pk_d = nc.dram_tensor("pk", [n_macros, P, nrg, G], f32,
                              kind="ExternalOutput")

    psm_bufs = (8 // nrg) if scheme == "c" else 2
    with tile.TileContext(nc) as tc:
        with tc.tile_pool(name="wpool", bufs=1) as wpool, \
             tc.tile_pool(name="xpool", bufs=6) as xpool, \
             tc.tile_pool(name="cspool", bufs=3) as cspool, \
             tc.tile_pool(name="vpool", bufs=3) as vpool, \
             tc.tile_pool(name="mpool", bufs=3) as mpool, \
             tc.tile_pool(name="psm", bufs=psm_bufs, space="PSUM") as psm, \
             tc.tile_pool(name="psc", bufs=2, space="PSUM") as psc:

            wh = wpool.tile([P, KC, DOUT], f16)
            nc.sync.dma_start(wh[:], wh_d[:])
            if scheme == "a":
                wl = wpool.tile([P, KC, DOUT], f16)
                nc.sync.dma_start(wl[:], wl_d[:])
            else:
                wh8 = wpool.tile([P, KC, DOUT], x8dt)
                nc.scalar.dma_start(wh8[:], wh8_d[:])
                wl8 = wpool.tile([P, KC, DOUT], x8dt)
                nc.gpsimd.dma_start(wl8[:], wl8_d[:])
            if scheme == "b":
                # fp16 identity for folding the correction PSUM into main
                ident = wpool.tile([P, P], f16)
                nc.gpsimd.memset(ident[:], 1.0)
                nc.gpsimd.affine_select(
                    ident[:], ident[:], pattern=[[-1, P]],
                    compare_op=mybir.AluOpType.is_equal, fill=0.0,
                    base=0, channel_multiplier=1)
            if with_bias:
                bh = wpool.tile([1, DOUT], f16)
                nc.sync.dma_start(bh[:], bh_d[:])
                bl = wpool.tile([1, DOUT], f16)
                nc.sync.dma_start(bl[:], bl_d[:])
                ones = wpool.tile([1, P], f16)
                nc.gpsimd.memset(ones[:], 1.0)

            if mask_mode not in ("cmp", "t1", "mix"):
                # lane-id pattern 0,1,2,3 repeating + the ~3 AND-mask scalar
                it = wpool.tile([P, 2 * DOUT], u32)
                nc.gpsimd.iota(it[:], pattern=[[0, 2 * G], [1, U]], base=0,
                               channel_multiplier=0)
                msk = wpool.tile([P, 1], u32)
                nc.vector.memset(msk[:], 0xFFFFFFFC)
            else:
                it = msk = None

            for mt in range(n_macros):
                if scheme == "a":
                    xh_t = xpool.tile([P, KC, MACRO], f16, tag="xh")
                    nc.sync.dma_start(xh_t[:], xh_d[mt, :, :, :])
                    xl_t = xpool.tile([P, KC, MACRO], f16, tag="xl")
                    nc.sync.dma_start(xl_t[:], xl_d[mt, :, :, :])

                    ps = psm.tile([P, 2 * DOUT], f32)
                    for s in range(2):
                        acc = ps[:, s * DOUT:(s + 1) * DOUT]
                        mms = []
                        if with_bias:
                            mms.append((ones[:, :], bh[:, :]))
                            mms.append((ones[:, :], bl[:, :]))
                        rs = slice(s * P, (s + 1) * P)
                        for (xa, wb) in ((xl_t, wh), (xh_t, wl), (xh_t, wh)):
                            for c in range(KC):
                                mms.append((xa[:, c, rs], wb[:, c, :]))
                        last = len(mms) - 1
                        for i, (lhsT, rhs) in enumerate(mms):
                            nc.tensor.matmul(acc, lhsT, rhs,
                                             start=(i == 0), stop=(i == last))
                elif scheme == "c":
                    xh_t = xpool.tile([P, KC, macro], f16, tag="xh")
                    nc.sync.dma_start(xh_t[:], xh_d[mt, :, :, :])
                    x8_t = xpool.tile([P, 2, KC, macro], mybir.dt.uint8,
                                      tag="x8")
                    nc.sync.dma_start(x8_t[:], x8_d[mt])
                    xh8_t = x8_t[:, 0].bitcast(x8dt)
                    xl8_t = x8_t[:, 1].bitcast(x8dt)

                    ps = psm.tile([P, nrg * DOUT], f32, tag="ps")
                    # all fp16 mains first, then all fp8 DR corrections:
                    # dtype switches flush the PE pipeline, so keep runs long
                    for s in range(nrg):
                        acc = ps[:, s * DOUT:(s + 1) * DOUT]
                        rs = slice(s * P, (s + 1) * P)
                        first = True
                        if with_bias:
                            nc.tensor.matmul(acc, ones[:, :], bh[:, :],
                                             start=True, stop=False)
                            nc.tensor.matmul(acc, ones[:, :], bl[:, :],
                                             start=False, stop=False)
                            first = False
                        for c in range(KC):
                            nc.tensor.matmul(acc, xh_t[:, c, rs], wh[:, c, :],
                                             start=first, stop=False)
                            first = False
                    for s in range(nrg):
                        acc = ps[:, s * DOUT:(s + 1) * DOUT]
                        rs = slice(s * P, (s + 1) * P)
                        # e5m2 corrections at natural scale, same accumulator
                        nc.tensor.matmul(acc, xl8_t[:, :, rs], wh8[:, :, :],
                                         start=False, stop=False, perf_mode=DR)
                        nc.tensor.matmul(acc, xh8_t[:, :, rs], wl8[:, :, :],
                                         start=False, stop=True, perf_mode=DR)
                else:
                    xh_t = xpool.tile([P, KC, MACRO], f16, tag="xh")
                    nc.sync.dma_start(xh_t[:], xh_d[mt, :, :, :])
                    xh8_t = xpool.tile([P, KC, MACRO], f8, tag="xh8")
                    nc.sync.dma_start(xh8_t[:], xh8_d[mt, :, :, :])
                    xl8_t = xpool.tile([P, KC, MACRO], f8, tag="xl8")
                    nc.sync.dma_start(xl8_t[:], xl8_d[mt, :, :, :])

                    cps = psc.tile([P, 2 * DOUT], f32)
                    ps = psm.tile([P, 2 * DOUT], f32)
                    # fp8 DoubleRow corrections first so the scalar engine can
                    # rescale them while the PE runs the fp16 main products
                    for s in range(2):
                        rs = slice(s * P, (s + 1) * P)
                        cacc = cps[:, s * DOUT:(s + 1) * DOUT]
                        nc.tensor.matmul(cacc, xl8_t[:, :, rs], wh8[:, :, :],
                                         start=True, stop=False, perf_mode=DR)
                        nc.tensor.matmul(cacc, xh8_t[:, :, rs], wl8[:, :, :],
                                         start=False, stop=True, perf_mode=DR)
                    csb = cspool.tile([P, 2, DOUT], f16)
                    for s in range(2):
                        nc.scalar.activation(
                            csb[:, s, :], cps[:, s * DOUT:(s + 1) * DOUT],
                            mybir.ActivationFunctionType.Copy, scale=SCORR)
                    for s in range(2):
                        rs = slice(s * P, (s + 1) * P)
                        acc = ps[:, s * DOUT:(s + 1) * DOUT]
                        first = True
                        if with_bias:
                            nc.tensor.matmul(acc, ones[:, :], bh[:, :],
                                             start=True, stop=False)
                            nc.tensor.matmul(acc, ones[:, :], bl[:, :],
                                             start=False, stop=False)
                            first = False
                        for c in range(KC):
                            nc.tensor.matmul(acc, xh_t[:, c, rs], wh[:, c, :],
                                             start=first, stop=False)
                            first = False
                    for s in range(2):
                        acc = ps[:, s * DOUT:(s + 1) * DOUT]
                        nc.tensor.matmul(acc, ident[:], csb[:, s, :],
                                         start=False, stop=True)

                if mask_mode == "cmp":
                    _mask_cmp_store(nc, vpool, mpool, ps, ob_d[mt], nrg)
                elif mask_mode == "t1":
                    _mask_t1_store(nc, vpool, mpool, ps, ob_d[mt], nrg)
                elif mask_mode == "mix":
                    if mt % 2 == 0:
                        _mask_t1_store(nc, vpool, mpool, ps, ob_d[mt // 2], nrg)
                    else:
                        _mask_cmp_store(nc, vpool, mpool, ps, ob2_d[mt // 2], nrg)
                else:
                    _mask_and_store(nc, vpool, mpool, it, msk, ps, pk_d[mt],
                                    mask_mode)

    nc.compile()
    return nc


_programs: dict = {}


def _get_program(n_macros: int, with_bias: bool, variant: str = "a"):
    key = (n_macros, with_bias, variant)
    if key not in _programs:
        _programs[key] = build_program(n_macros, with_bias, variant)
    return _programs[key]


def _split_fp16(a: np.ndarray):
    hi = a.astype(np.float16)
    lo = (a - hi.astype(np.float32)).astype(np.float16)
    return hi, lo


def _pack_b(b: np.ndarray):
    """[DOUT] fp32 -> (hi, lo) [1, DOUT] fp16."""
    return _split_fp16(np.ascontiguousarray(b.astype(np.float32).reshape(1, DOUT)))


def _tile_x(a: np.ndarray, n_macros: int, macro: int = MACRO) -> np.ndarray:
    """[rows, DIN] -> [n_macros, P, KC, macro] keeping dtype."""
    at = np.ascontiguousarray(a.T)                      # [DIN, rows]
    at = at.reshape(KC, P, n_macros, macro)             # [c, p, mt, r]
    return np.ascontiguousarray(at.transpose(2, 1, 0, 3))


def _pack_x(xs: np.ndarray, n_macros: int):
    """[rows, DIN] fp32 -> (hi, lo) tiled [n_macros, P, KC, MACRO] fp16."""
    hi, lo = _split_fp16(xs)
    return [_tile_x(a, n_macros) for a in (hi, lo)]


def _pack_x8(xs: np.ndarray, n_macros: int):
    """fp32 rows -> (xh fp16, xh8 fp8, xl8 fp8 scaled) tiles for variant b."""
    f8np = mybir.dt.np(f8)
    hi = xs.astype(np.float16)
    lo32 = xs - hi.astype(np.float32)
    xh = _tile_x(hi, n_macros)
    xh8 = _tile_x(hi.astype(f8np), n_macros)
    xl8 = _tile_x((lo32 * SXL).astype(f8np), n_macros)
    return xh, xh8, xl8


def _pack_x8e5(xs: np.ndarray, n_macros: int, macro: int = MACRO):
    """fp32 rows -> (xh fp16, x8 packed u8 [nm, P, 2, KC, MACRO]), variant c.

    x8 piece 0 = e5m2(xh / 2^8), piece 1 = e5m2(xl * 2^8).
    """
    e5 = mybir.dt.np(f8e5)
    hi = xs.astype(np.float16)
    lo32 = xs - hi.astype(np.float32)
    xh = _tile_x(hi, n_macros, macro)
    xh8 = _tile_x((hi.astype(np.float32) / SC).astype(e5), n_macros, macro)
    xl8 = _tile_x((lo32 * SC).astype(e5), n_macros, macro)
    x8 = np.stack([xh8.view(np.uint8), xl8.view(np.uint8)], axis=2)
    return xh, np.ascontiguousarray(x8)


# physical output column c -> logical unit index (4*g + u), u = 2*b1 + b0,
# c = b0*256 + b1*128 + g: makes all LWTA compare partners contiguous blocks
LPERM = np.array([4 * (c & 127) + 2 * ((c >> 7) & 1) + (c >> 8)
                  for c in range(DOUT)])


def _pack_w8e5(W: np.ndarray, perm: bool = False):
    """-> (wh fp16, wh8 e5m2/2^8, wl8 e5m2*2^8) tiles, variant c."""
    e5 = mybir.dt.np(f8e5)
    wT = np.ascontiguousarray(W.astype(np.float32).T)   # [DIN, DOUT]
    if perm:
        wT = np.ascontiguousarray(wT[:, LPERM])
    hi = wT.astype(np.float16)
    lo32 = wT - hi.astype(np.float32)
    wh = _tile_w(hi)
    wh8 = _tile_w((wT / SC).astype(e5))
    wl8 = _tile_w((lo32 * SC).astype(e5))
    return wh, wh8, wl8


def _tile_w(a: np.ndarray) -> np.ndarray:
    """[DIN, DOUT] -> [P, KC, DOUT] keeping dtype."""
    return np.ascontiguousarray(a.reshape(KC, P, DOUT).transpose(1, 0, 2))


def _pack_w(W: np.ndarray):
    """[DOUT, DIN] fp32 -> (hi, lo) tiled [P, KC, DOUT] fp16 of W.T."""
    wT = W.astype(np.float32).T                         # [DIN, DOUT]
    hi, lo = _split_fp16(np.ascontiguousarray(wT))
    return [_tile_w(a) for a in (hi, lo)]


def _pack_w8(W: np.ndarray):
    """-> (wh fp16, wh8 fp8 * 2^6, wl8 fp8 * 2^17) tiles for variant b."""
    f8np = mybir.dt.np(f8)
    wT = np.ascontiguousarray(W.astype(np.float32).T)   # [DIN, DOUT]
    hi = wT.astype(np.float16)
    lo32 = wT - hi.astype(np.float32)
    wh = _tile_w(hi)
    wh8 = _tile_w((wT * SWH).astype(f8np))
    wl8 = _tile_w((lo32 * SWL).astype(f8np))
    return wh, wh8, wl8


def _unpack_result_map(rm: dict, rows: int) -> np.ndarray:
    """Unpack one core's result dict (handles the mix-mode tensor pair)."""
    if "pk2" in rm:
        t1, cm = rm["pk"], rm["pk2"]
        nm = t1.shape[0] + cm.shape[0]
        macro = rows // nm
        a = _unpack_out_t1(t1, t1.shape[0] * macro)
        b = _unpack_out_cmp(cm, cm.shape[0] * macro)
        out = np.empty((nm, macro, DOUT), np.float32)
        out[0::2] = a.reshape(t1.shape[0], macro, DOUT)
        out[1::2] = b.reshape(cm.shape[0], macro, DOUT)
        return out.reshape(rows, DOUT)
    return _unpack_out(rm["pk"], rows)


def _unpack_out(pk: np.ndarray, rows: int) -> np.ndarray:
    """Device output -> dense [rows, DOUT] f32 (bit-packed, cmp, t1 format)."""
    if pk.dtype == np.uint8:
        if pk.shape[2] % 1280 == 0:
            return _unpack_out_t1(pk, rows)
        return _unpack_out_cmp(pk, rows)
    # row = mt*256 + s*128 + p  ->  [mt, s, p, g]
    m2 = np.ascontiguousarray(pk.transpose(0, 2, 1, 3)).reshape(rows, G)
    iv = m2.view(np.uint32)
    idx = (iv & np.uint32(3)).astype(np.int64)
    val = (iv & np.uint32(0xFFFFFFFC)).view(np.float32)
    out = np.zeros((rows, G, U), dtype=np.float32)
    np.put_along_axis(out, idx[:, :, None], val[:, :, None], axis=2)
    return out.reshape(rows, DOUT)


def _unpack_out_t1(ob: np.ndarray, rows: int) -> np.ndarray:
    """[n_macros, P, nh*(t1 f32 | d1 u8)] -> [rows, DOUT].

    Per half-block order (s_local, b1, g); host computes the level-2 max.
    """
    nm = ob.shape[0]
    nrg = ob.shape[2] // 1280
    nh = 2 if nrg == 4 else 1
    rgh = nrg // nh
    eh = rgh * DOUT
    blk = ob.reshape(nm, P, nh, eh * 2 + eh // 2)
    t1 = blk[:, :, :, 0:eh * 2].copy().view(np.float32)
    t1 = t1.reshape(nm, P, nrg, 2, G)
    d1 = (blk[:, :, :, eh * 2:] != 0).reshape(nm, P, nrg, 2, G)
    b1w = np.where(t1[:, :, :, 0, :] >= t1[:, :, :, 1, :], 0, 1)
    val = np.maximum(t1[:, :, :, 0, :], t1[:, :, :, 1, :])
    b0w = np.where(np.take_along_axis(d1, b1w[:, :, :, None, :],
                                      axis=3)[:, :, :, 0, :], 0, 1)
    idx = (2 * b1w + b0w).astype(np.int64)
    val = val.transpose(0, 2, 1, 3).reshape(rows, G)
    idx = idx.transpose(0, 2, 1, 3).reshape(rows, G)
    out = np.zeros((rows, G, U), dtype=np.float32)
    np.put_along_axis(out, idx[:, :, None], val[:, :, None], axis=2)
    return out.reshape(rows, DOUT)


def _unpack_out_cmp(ob: np.ndarray, rows: int) -> np.ndarray:
    """[n_macros, P, nh*(m2 f32 | d1 u8 | d2 u8)] -> [rows, DOUT].

    Per half-block: d1 order (s_local, b1, g), m2/d2 order (s_local, g); a
    set compare bit means the first (lower-index) operand won.
    """
    nm = ob.shape[0]
    nrg = ob.shape[2] // 896
    nh = 2 if nrg == 4 else 1
    rgh = nrg // nh
    eh = rgh * DOUT
    blk = ob.reshape(nm, P, nh, eh + eh // 2 + eh // 4)
    val = blk[:, :, :, 0:eh].copy().view(np.float32).reshape(nm, P, nrg, G)
    d1 = (blk[:, :, :, eh:eh + eh // 2] != 0).reshape(nm, P, nrg, 2, G)
    d2 = (blk[:, :, :, eh + eh // 2:] != 0).reshape(nm, P, nrg, G)
    b1w = np.where(d2, 0, 1)
    b0w = np.where(np.take_along_axis(d1, b1w[:, :, :, None, :],
                                      axis=3)[:, :, :, 0, :], 0, 1)
    idx = (2 * b1w + b0w).astype(np.int64)
    # row = mt*macro + s*128 + p
    val = val.transpose(0, 2, 1, 3).reshape(rows, G)
    idx = idx.transpose(0, 2, 1, 3).reshape(rows, G)
    out = np.zeros((rows, G, U), dtype=np.float32)
    np.put_along_axis(out, idx[:, :, None], val[:, :, None], axis=2)
    return out.reshape(rows, DOUT)


def _build_in_maps(x, W, b, with_bias, n_macros, variant):
    in_maps = []
    scheme = variant[0]
    if scheme == "a":
        wh, wl = _pack_w(W)
        for i in range(NCORES):
            xh, xl = _pack_x(x[i * ROWS:(i + 1) * ROWS], n_macros)
            im = {"xh": xh, "xl": xl, "wh": wh, "wl": wl}
            if with_bias:
                im["bh"], im["bl"] = _pack_b(b)
            in_maps.append(im)
    elif scheme == "b":
        wh, wh8, wl8 = _pack_w8(W)
        for i in range(NCORES):
            xh, xh8, xl8 = _pack_x8(x[i * ROWS:(i + 1) * ROWS], n_macros)
            im = {"xh": xh, "xh8": xh8, "xl8": xl8,
                  "wh": wh, "wh8": wh8, "wl8": wl8}
            if with_bias:
                im["bh"], im["bl"] = _pack_b(b)
            in_maps.append(im)
    else:
        macro = 512 if variant.endswith("5") else MACRO
        n_macros = ROWS // macro
        perm = ("_cmp" in variant or "_t1" in variant
                or "_mix" in variant)
        bp = b[LPERM] if perm else b
        wh, wh8, wl8 = _pack_w8e5(W, perm=perm)
        for i in range(NCORES):
            xh, x8 = _pack_x8e5(x[i * ROWS:(i + 1) * ROWS], n_macros, macro)
            im = {"xh": xh, "x8": x8, "wh": wh, "wh8": wh8, "wl8": wl8}
            if with_bias:
                im["bh"], im["bl"] = _pack_b(bp)
            in_maps.append(im)
    return in_maps


VARIANT = "c_mix5"


def n_macros_for(variant: str) -> int:
    return ROWS // (512 if variant.endswith("5") else MACRO)


def kernel(x: np.ndarray, W: np.ndarray, b: np.ndarray) -> np.ndarray:
    x = np.asarray(x, dtype=np.float32)
    W = np.asarray(W, dtype=np.float32)
    b = np.asarray(b, dtype=np.float32)
    assert x.shape == (N, DIN) and W.shape == (DOUT, DIN) and b.shape == (DOUT,)

    with_bias = bool(np.any(b))
    n_macros = n_macros_for(VARIANT)
    nc = _get_program(n_macros, with_bias, VARIANT)

    in_maps = _build_in_maps(x, W, b, with_bias, n_macros, VARIANT)
    res = run_bass_kernel_spmd(nc, in_maps, list(range(NCORES)))
    return np.concatenate(
        [_unpack_result_map(res.results[i], ROWS) for i in range(NCORES)], axis=0)
